# revision 26
# baseline (speedup 1.0000x reference)
"""Trainium2 Bass kernel for nn_MFF_38809324487316 (topk_masking).

Strategy (pure data parallel, batch sharded 16 -> 8 cores x 2 samples):
  Per sample, the whole gather/mean/1x1-conv pipeline is reformulated as
  one data-dependent [256,256] @ [256,6400] matmul:
    rows   0..127 : G (+I)   -> x1[pos_idx] + x1          (tmp1 gather, x1-add folded)
    row    128    : negmask/128 (+e128)                   (mean of negatives + x1)
    rows 129..255 : Wg = W_pos @ G + outer(w_last, negmask/128)   (the 1x1 conv)
  followed by BN+LeakyReLU (one fused Lrelu ACT op with per-partition
  alpha/scale/bias; the mean row passes through via alpha=1) + x1-add.
  The 0/1 matrices are built on-device from the ECA channel scores via
  pairwise-comparison ranking (no sort, no data-dependent control flow).

  The big matmuls run in float32r (1 cycle/row on the PE at free-dim
  >=256, vs 4 cycles/row for plain fp32); the tiny ranking matmuls stay
  fp32 since reduced precision there could flip top-k membership of
  near-tied channels.

  build_nc(niter=N) wraps the whole per-iteration pipeline (x1 load ->
  ranking -> matmuls -> stores -> x0 passthrough) in a hardware For_i
  loop; test.py times niter=1 vs niter=N and reports the marginal
  per-iteration time, cancelling the ~87 ms axon-tunnel dispatch RTT
  that would otherwise swamp the ~0.2 ms kernel.
"""

import sys

sys.path.insert(0, "/opt/trn_rl_repo")

import numpy as np

import concourse.bass as bass
import concourse.tile as tile
from concourse import mybir
from concourse.bass_utils import run_bass_kernel_spmd

B, C, H, W = 16, 256, 80, 80
HALF = C // 2          # 128
NPIX = H * W           # 6400
NCORES = 8
SPC = B // NCORES      # 2 samples per core
NT = 512               # matmul n-tile (one PSUM bank of f32)
BN_EPS = 1e-5
F32 = mybir.dt.float32
F32R = mybir.dt.float32r


def build_nc(npix=NPIX, nsamp=SPC, niter=1):
    nc = bass.Bass("TRN2", target_bir_lowering=False, debug=False)

    def din(name, shape):
        return nc.dram_tensor(name, shape, F32, kind="ExternalInput").ap()

    x0 = din("x0", [nsamp, C, npix])
    x1 = din("x1", [nsamp, C, npix])
    wposT = din("wposT", [HALF, HALF - 1])      # W_pos^T
    wlastb = din("wlastb", [HALF, HALF - 1])    # w_last broadcast over partitions
    bnA = din("bnA", [HALF, 1])                 # [0]=1, [p]=gamma/sqrt(var+eps) for row p-1
    bnB = din("bnB", [HALF, 1])                 # [0]=0, [p]=beta-mean*bnA
    alphav = din("alphav", [HALF, 1])           # [0]=1 (identity), else 0.1 (lrelu slope)
    ecaw = din("ecaw", [1, 5])
    id128 = din("id128", [HALF, HALF])
    tri = din("tri", [HALF, HALF])              # tri[k,j] = 1 if k<j
    onesm = din("onesm", [HALF, HALF])
    iota = din("iota", [HALF, HALF])            # iota[j,p] = p
    ones1r = din("ones1r", [1, HALF])
    out = nc.dram_tensor("out", [nsamp, 2 * C, npix], F32, kind="ExternalOutput").ap()

    ntl = []
    o = 0
    while o < npix:
        ntl.append((o, min(NT, npix - o)))
        o += NT

    from contextlib import ExitStack
    with tile.TileContext(nc) as tc, ExitStack() as st:
        consts = st.enter_context(tc.tile_pool(name="consts", bufs=1))
        xin = st.enter_context(tc.tile_pool(name="xin", bufs=1))
        lhp = st.enter_context(tc.tile_pool(name="lhp", bufs=4))
        gtp = st.enter_context(tc.tile_pool(name="gtp", bufs=2))
        misc = st.enter_context(tc.tile_pool(name="misc", bufs=2))
        obp = st.enter_context(tc.tile_pool(name="obp", bufs=2))
        pbig = st.enter_context(tc.tile_pool(name="pbig", bufs=4, space="PSUM"))
        pmisc = st.enter_context(tc.tile_pool(name="pmisc", bufs=3, space="PSUM"))

        # ---- constants into SBUF (outside the timing loop) ----
        c_id = consts.tile([HALF, HALF], F32)
        c_tri = consts.tile([HALF, HALF], F32)
        c_ones = consts.tile([HALF, HALF], F32)
        c_iota = consts.tile([HALF, HALF], F32)
        c_1r = consts.tile([1, HALF], F32)
        c_wposT = consts.tile([HALF, HALF - 1], F32)
        c_wlastb = consts.tile([HALF, HALF - 1], F32)
        c_bnA = consts.tile([HALF, 1], F32)
        c_bnB = consts.tile([HALF, 1], F32)
        c_alpha = consts.tile([HALF, 1], F32)
        c_ecaw = consts.tile([1, 5], F32)
        for t, d in (
            (c_id, id128), (c_tri, tri), (c_ones, onesm), (c_iota, iota),
            (c_1r, ones1r), (c_wposT, wposT), (c_wlastb, wlastb),
            (c_bnA, bnA), (c_bnB, bnB), (c_alpha, alphav),
            (c_ecaw, ecaw),
        ):
            nc.gpsimd.dma_start(out=t, in_=d)

        def body():
            # ---- x1 into SBUF (stays resident for both passes) ----
            # All in-loop DMAs ride the two HWDGE rings: SWDGE (gpsimd)
            # DMAs inside a For_i loop emit InstIncSwdgeSem ISA ops that
            # this walrus cannot encode ("ISA wrong length").
            # Independent transfers (x1 loads, x0 passthrough) go on the
            # sync ring; the compute-dependent ob stores go on the scalar
            # ring so their semaphore waits can't head-of-line-block the
            # loads at the sequencer.
            X = [[None, None] for _ in range(nsamp)]
            for s in range(nsamp):
                for h in range(2):
                    t = xin.tile([HALF, npix], F32, tag=f"x_{s}_{h}")
                    nc.sync.dma_start(
                        out=t, in_=x1[s, h * HALF:(h + 1) * HALF, :])
                    X[s][h] = t

            # ---- phase A: channel sums -> row layout R [1, nsamp*(C+4)] ----
            R = misc.tile([1, nsamp * (C + 4)], F32, tag="R", bufs=1)
            nc.vector.memset(R, 0.0)
            for s in range(nsamp):
                for h in range(2):
                    sm = misc.tile([HALF, 1], F32, tag=f"sums_{s}_{h}", bufs=1)
                    nc.vector.reduce_sum(out=sm, in_=X[s][h],
                                         axis=mybir.AxisListType.X)
                    pr = pmisc.tile([1, HALF], F32, tag="mp")
                    nc.tensor.matmul(pr, sm, c_id, start=True, stop=True)
                    nc.scalar.copy(
                        out=R[0:1, s * (C + 4) + 2 + h * HALF:
                              s * (C + 4) + 2 + (h + 1) * HALF],
                        in_=pr)

            # ---- ECA conv k=5 along channels: Y [1, nsamp*C] ----
            Yr = misc.tile([1, nsamp, C], F32, tag="Y", bufs=1)
            Rr = R.rearrange("p (s c) -> p s c", s=nsamp)
            nc.vector.tensor_scalar(
                out=Yr, in0=Rr[:, :, 0:C], scalar1=c_ecaw[0:1, 0:1],
                scalar2=None, op0=mybir.AluOpType.mult)
            for k in range(1, 5):
                nc.vector.scalar_tensor_tensor(
                    out=Yr, in0=Rr[:, :, k:k + C], scalar=c_ecaw[0:1, k:k + 1],
                    in1=Yr, op0=mybir.AluOpType.mult, op1=mybir.AluOpType.add)

            # ---- per-sample: rank -> masks -> Lh ----
            LHS = [[None, None] for _ in range(nsamp)]
            for s in range(nsamp):
                pb = pmisc.tile([HALF, C], F32, tag="mp")
                nc.tensor.matmul(pb, c_1r, Yr[0:1, s, :], start=True, stop=True)
                m_sb, negdiv = [], []
                for h in range(2):
                    pyc = pmisc.tile([HALF, 1], F32, tag="mp")
                    nc.tensor.matmul(
                        pyc, Yr[0:1, s, h * HALF:(h + 1) * HALF],
                        c_1r[0:1, 0:1], start=True, stop=True)
                    yc = misc.tile([HALF, 1], F32, tag="yc")
                    nc.vector.tensor_copy(out=yc, in_=pyc)
                    cmp = misc.tile([HALF, C], F32, tag="cmp")
                    nc.vector.tensor_scalar(
                        out=cmp, in0=pb, scalar1=yc, scalar2=None,
                        op0=mybir.AluOpType.is_gt)
                    rankd = misc.tile([HALF, 1], F32, tag="rankd")
                    nc.vector.reduce_sum(out=rankd, in_=cmp,
                                         axis=mybir.AxisListType.X)
                    mh = misc.tile([HALF, 1], F32, tag="m")
                    nc.vector.tensor_scalar(
                        out=mh, in0=rankd, scalar1=float(HALF), scalar2=None,
                        op0=mybir.AluOpType.is_lt)
                    # nd = (1 - mh)/128  (mean-of-negatives divisor; also
                    # reused as the not-selected mask via nd*128)
                    nd = misc.tile([HALF, 1], F32, tag="nd")
                    nc.vector.tensor_scalar(
                        out=nd, in0=mh, scalar1=-1.0 / HALF,
                        scalar2=1.0 / HALF, op0=mybir.AluOpType.mult,
                        op1=mybir.AluOpType.add)
                    m_sb.append(mh); negdiv.append(nd)

                pr0 = pmisc.tile([HALF, 1], F32, tag="mp")
                nc.tensor.matmul(pr0, c_tri, m_sb[0], start=True, stop=True)
                pr1 = pmisc.tile([HALF, 1], F32, tag="mp")
                nc.tensor.matmul(pr1, c_ones, m_sb[0], start=True, stop=False)
                nc.tensor.matmul(pr1, c_tri, m_sb[1], start=False, stop=True)
                for h, prh in ((0, pr0), (1, pr1)):
                    rp = misc.tile([HALF, 1], F32, tag="rp")
                    nc.vector.scalar_tensor_tensor(
                        out=rp, in0=negdiv[h], scalar=256.0 * HALF, in1=prh,
                        op0=mybir.AluOpType.mult, op1=mybir.AluOpType.add)
                    gt_sb = gtp.tile([HALF, HALF], F32, tag="gt")
                    nc.vector.tensor_scalar(
                        out=gt_sb, in0=c_iota, scalar1=rp, scalar2=None,
                        op0=mybir.AluOpType.is_equal)
                    pgm = pmisc.tile([HALF, HALF], F32, tag="mp")
                    nc.tensor.matmul(pgm, gt_sb, c_id, start=True, stop=True)
                    gm_sb = gtp.tile([HALF, HALF], F32, tag="gm")
                    nc.vector.tensor_copy(out=gm_sb, in_=pgm)
                    pwg = pmisc.tile([HALF, HALF - 1], F32, tag="mp")
                    nc.tensor.matmul(pwg, gm_sb, c_wposT, start=True, stop=True)
                    lh = lhp.tile([HALF, C], F32, tag="lh")
                    if h == 0:
                        nc.vector.tensor_add(
                            out=lh[:, 0:HALF], in0=gt_sb, in1=c_id)
                    else:
                        nc.vector.tensor_copy(out=lh[:, 0:HALF], in_=gt_sb)
                    nc.vector.tensor_copy(
                        out=lh[:, HALF:HALF + 1], in_=negdiv[h])
                    nc.vector.scalar_tensor_tensor(
                        out=lh[:, HALF + 1:C], in0=c_wlastb, scalar=negdiv[h],
                        in1=pwg, op0=mybir.AluOpType.mult,
                        op1=mybir.AluOpType.add)
                    LHS[s][h] = lh
                # NOTE: no identity fold for inner row 128 — the mh1 epilogue
                # adds x1 for all 128 partitions (incl. the mean row at p=0).

            # f32r copies of the LHS matrices (fp32 matmul is 4 cycles/row
            # on the PE; f32r with free dim >=256 is 1 cycle/row). The DVE
            # copy is the sanctioned f32r-rounding producer.
            LHR = [[None, None] for _ in range(nsamp)]
            for s in range(nsamp):
                for h in range(2):
                    lhr = lhp.tile([HALF, C], F32R, tag="lhr")
                    nc.vector.tensor_copy(out=lhr, in_=LHS[s][h])
                    LHR[s][h] = lhr

            # ---- big matmuls + epilogue + stores ----
            # Loop tiles outermost so each x1 slice is rounded to f32r once
            # and feeds both output halves (mh=0 gather rows, mh=1 conv rows).
            SPL = 3072  # tile-aligned store split: write back first half early
            for s in range(nsamp):
                ob0 = obp.tile([HALF, npix], F32, tag="ob0", bufs=1)
                ob1 = obp.tile([HALF, npix], F32, tag="ob1", bufs=1)
                for (n0, nsz) in ntl:
                    xs = [None, None]
                    for h in range(2):
                        xs[h] = misc.tile([HALF, NT], F32R, tag=f"xs{h}",
                                          bufs=3, name=f"xs{h}")
                        nc.vector.tensor_copy(
                            out=xs[h][:, :nsz], in_=X[s][h][:, n0:n0 + nsz])
                    for mh in range(2):
                        ps = pbig.tile([HALF, NT], F32, tag="pb")
                        nc.tensor.matmul(
                            ps[:, :nsz],
                            LHR[s][0][:, mh * HALF:(mh + 1) * HALF],
                            xs[0][:, :nsz], start=True, stop=False)
                        nc.tensor.matmul(
                            ps[:, :nsz],
                            LHR[s][1][:, mh * HALF:(mh + 1) * HALF],
                            xs[1][:, :nsz], start=False, stop=True)
                        if mh == 0:
                            nc.vector.tensor_copy(
                                out=ob0[:, n0:n0 + nsz], in_=ps[:, :nsz])
                        else:
                            # BN + LeakyReLU(0.1) fused into one ACT op:
                            # prelu(ps*bnA + bnB; alpha) with alpha[0]=1 so the
                            # mean row (p=0) passes through unchanged. (Lrelu's
                            # alpha operand is ignored on this HW — fixed 0.01
                            # slope; Prelu honors the per-partition alpha AP.)
                            ta = misc.tile([HALF, NT], F32, tag="ta", bufs=3)
                            nc.scalar.activation(
                                out=ta[:, :nsz], in_=ps[:, :nsz],
                                func=mybir.ActivationFunctionType.Prelu,
                                bias=c_bnB, scale=c_bnA, alpha=c_alpha)
                            nc.vector.tensor_add(
                                out=ob1[:, n0:n0 + nsz], in0=ta[:, :nsz],
                                in1=X[s][1][:, n0:n0 + nsz])
                    if n0 + nsz == SPL:
                        nc.scalar.dma_start(
                            out=out[s, C:C + HALF, 0:SPL], in_=ob0[:, 0:SPL])
                        nc.scalar.dma_start(
                            out=out[s, C + HALF:2 * C, 0:SPL],
                            in_=ob1[:, 0:SPL])
                nc.scalar.dma_start(
                    out=out[s, C:C + HALF, SPL:npix], in_=ob0[:, SPL:npix])
                nc.scalar.dma_start(
                    out=out[s, C + HALF:2 * C, SPL:npix], in_=ob1[:, SPL:npix])
                # x0 passthrough DRAM->DRAM (sync HWDGE ring, no deps)
                nc.sync.dma_start(out=out[s, 0:C, :], in_=x0[s, :, :])

        if niter == 1:
            body()
        else:
            with tc.For_i(0, niter, 1):
                body()
    return nc


def _split_multiwait_drains(nc):
    """This container's walrus rejects >1 sync-wait on one instruction
    ("Too many sync wait commands" in setupSyncWait). Tile's kernel-tail
    Drain carries one wait per outstanding semaphore — split it into a
    chain of single-wait Drains."""
    for fn in nc.m.functions:
        for blk in fn.blocks:
            insts = list(blk.instructions)
            changed = False
            out = []
            for inst in insts:
                si = getattr(inst, "sync_info", None)
                waits = list(si.on_wait) if (si and si.on_wait) else []
                if len(waits) > 1:
                    for j, w in enumerate(waits[:-1]):
                        nd = mybir.InstEventSemaphore(
                            name=f"{inst.name}-sw{j}", ins=[], outs=[])
                        nd.engine = inst.engine
                        nd.sync_info = mybir.SyncInfo(
                            on_wait=[w], on_update=[])
                        out.append(nd)
                    si.on_wait = [waits[-1]]
                    changed = True
                out.append(inst)
            if changed:
                blk.instructions = out
    return nc


def host_consts(conv_w, bn_gamma, bn_beta, bn_mean, bn_var, eca_w):
    conv_w = np.asarray(conv_w, np.float32)
    a = (np.asarray(bn_gamma, np.float64)
         / np.sqrt(np.asarray(bn_var, np.float64) + BN_EPS))
    bnA = np.zeros((HALF, 1), np.float32)
    bnB = np.zeros((HALF, 1), np.float32)
    bnA[0, 0] = 1.0
    bnA[1:, 0] = a.astype(np.float32)
    bnB[1:, 0] = (np.asarray(bn_beta, np.float64)
                  - np.asarray(bn_mean, np.float64) * a).astype(np.float32)
    alphav = np.full((HALF, 1), 0.1, np.float32)
    alphav[0, 0] = 1.0
    return {
        "alphav": alphav,
        "wposT": np.ascontiguousarray(conv_w[:, :HALF].T),
        "wlastb": np.ascontiguousarray(
            np.tile(conv_w[:, HALF][None, :], (HALF, 1))),
        "bnA": bnA,
        "bnB": bnB,
        "ecaw": np.asarray(eca_w, np.float32).reshape(1, 5),
        "id128": np.eye(HALF, dtype=np.float32),
        "tri": np.triu(np.ones((HALF, HALF), np.float32), 1),
        "onesm": np.ones((HALF, HALF), np.float32),
        "iota": np.tile(np.arange(HALF, dtype=np.float32), (HALF, 1)),
        "ones1r": np.ones((1, HALF), np.float32),
    }


def kernel(x0, x1, eca_w, conv_w, bn_gamma, bn_beta, bn_mean, bn_var,
           _trace=False):
    x0 = np.asarray(x0, np.float32).reshape(B, C, NPIX)
    x1 = np.asarray(x1, np.float32).reshape(B, C, NPIX)
    cst = host_consts(conv_w, bn_gamma, bn_beta, bn_mean, bn_var, eca_w)
    nc = _split_multiwait_drains(build_nc())
    in_maps = []
    for c in range(NCORES):
        m = dict(cst)
        m["x0"] = np.ascontiguousarray(x0[c * SPC:(c + 1) * SPC])
        m["x1"] = np.ascontiguousarray(x1[c * SPC:(c + 1) * SPC])
        in_maps.append(m)
    res = run_bass_kernel_spmd(nc, in_maps, list(range(NCORES)), trace=False)
    out = np.concatenate([res.results[c]["out"] for c in range(NCORES)], axis=0)
    out = out.reshape(B, 2 * C, H, W)
    return out


def _make_jit_fn(nc):
    """jit(shard_map(bass_exec)) over 8 cores with donated zero output
    buffers, mirroring bass2jax.run_bass_via_pjrt but reusable."""
    import jax
    import jax.numpy as jnp
    from jax.sharding import Mesh, PartitionSpec
    from jax.experimental.shard_map import shard_map
    from concourse import bass2jax
    from concourse import mybir as _mb

    bass2jax.install_neuronx_cc_hook()
    pid_name = (nc.partition_id_tensor.name
                if nc.partition_id_tensor else None)
    in_names, out_names, out_avals, zero_shapes = [], [], [], []
    for alloc in nc.m.functions[0].allocations:
        if not isinstance(alloc, _mb.MemoryLocationSet):
            continue
        name = alloc.memorylocations[0].name
        if alloc.kind == "ExternalInput":
            if name != pid_name:
                in_names.append(name)
        elif alloc.kind == "ExternalOutput":
            out_names.append(name)
            shape = tuple(alloc.tensor_shape)
            dtype = _mb.dt.np(alloc.dtype)
            out_avals.append(jax.core.ShapedArray(shape, dtype))
            zero_shapes.append((shape, dtype))
    n_params = len(in_names)
    n_outs = len(out_names)
    all_names = list(in_names) + list(out_names) + (
        [pid_name] if pid_name else [])
    donate = tuple(range(n_params, n_params + n_outs))

    def _body(*args):
        operands = list(args)
        if pid_name:
            operands.append(bass2jax.partition_id_tensor())
        outs = bass2jax._bass_exec_p.bind(
            *operands, out_avals=tuple(out_avals), in_names=tuple(all_names),
            out_names=tuple(out_names), lowering_input_output_aliases=(),
            sim_require_finite=True, sim_require_nnan=True, nc=nc)
        return tuple(outs)

    devices = jax.devices()[:NCORES]
    mesh = Mesh(np.asarray(devices), ("core",))
    fn = jax.jit(shard_map(
        _body, mesh=mesh,
        in_specs=(PartitionSpec("core"),) * (n_params + n_outs),
        out_specs=(PartitionSpec("core"),) * n_outs,
        check_rep=False), donate_argnums=donate, keep_unused=True)
    sharding = jax.sharding.NamedSharding(mesh, PartitionSpec("core"))
    zfn = jax.jit(
        lambda: tuple(jnp.zeros((NCORES * sh[0], *sh[1:]), dt)
                      for (sh, dt) in zero_shapes),
        out_shardings=(sharding,) * n_outs)
    return fn, zfn, sharding, in_names, out_names


def _time_fn(fn, zfn, dev_in, iters=10, warmup=3):
    import time
    import jax
    times = []
    for i in range(warmup + iters):
        z = zfn()
        jax.block_until_ready(z)
        t0 = time.perf_counter()
        r = fn(*dev_in, *z)
        jax.block_until_ready(r)
        dt_s = time.perf_counter() - t0
        if i >= warmup:
            times.append(dt_s)
        del r
    times.sort()
    return times


NBIG = 513  # loop count of the timing NEFF


def bench(x0, x1, eca_w, conv_w, bn_gamma, bn_beta, bn_mean, bn_var,
          iters=14, warmup=3, nbig=NBIG):
    """Returns (output, per_iter_seconds, info). The per-call wall time
    through the axon tunnel is ~87 ms of dispatch RTT regardless of
    kernel content (a trivial copy kernel measures the same), so the
    kernel's HW execution time is measured differentially: a second
    NEFF runs the identical workload `nbig` times in a hardware For_i
    loop, and per-iteration time = (T(nbig) - T(1)) / (nbig - 1)."""
    import jax

    x0 = np.asarray(x0, np.float32).reshape(B, C, NPIX)
    x1 = np.asarray(x1, np.float32).reshape(B, C, NPIX)
    cst = host_consts(conv_w, bn_gamma, bn_beta, bn_mean, bn_var, eca_w)

    nc1 = _split_multiwait_drains(build_nc(niter=1))
    fn1, zfn1, sharding, in_names, out_names = _make_jit_fn(nc1)

    per_core = []
    for c in range(NCORES):
        m = dict(cst)
        m["x0"] = np.ascontiguousarray(x0[c * SPC:(c + 1) * SPC])
        m["x1"] = np.ascontiguousarray(x1[c * SPC:(c + 1) * SPC])
        per_core.append(m)
    concat_in = [np.concatenate([per_core[c][n] for c in range(NCORES)], axis=0)
                 for n in in_names]

    # correctness result from the niter=1 NEFF
    outs = fn1(*concat_in, *zfn1())
    jax.block_until_ready(outs)
    oidx = out_names.index("out")
    full = np.asarray(outs[oidx]).reshape(NCORES, SPC, 2 * C, NPIX)
    result = full.reshape(B, 2 * C, H, W)

    dev_in = [jax.device_put(a, sharding) for a in concat_in]
    t1 = _time_fn(fn1, zfn1, dev_in, iters=iters, warmup=warmup)

    ncN = _split_multiwait_drains(build_nc(niter=nbig))
    fnN, zfnN, _, _, _ = _make_jit_fn(ncN)
    outsN = fnN(*concat_in, *zfnN())
    jax.block_until_ready(outsN)
    fullN = np.asarray(outsN[oidx]).reshape(NCORES, SPC, 2 * C, NPIX)
    resultN = fullN.reshape(B, 2 * C, H, W)
    loop_dev = float(np.abs(resultN - result).max())

    tN = _time_fn(fnN, zfnN, dev_in, iters=iters, warmup=warmup)

    med1 = t1[len(t1) // 2]
    medN = tN[len(tN) // 2]
    per_iter = (medN - med1) / (nbig - 1)
    info = {
        "t1": t1, "tN": tN, "med1": med1, "medN": medN,
        "nbig": nbig, "loop_output_absdev": loop_dev,
    }
    return result, per_iter, info


# revision 27
# speedup vs baseline: 1.0130x; 1.0130x over previous
"""Trainium2 Bass kernel for nn_MFF_38809324487316 (topk_masking).

Strategy (pure data parallel, batch sharded 16 -> 8 cores x 2 samples):
  Per sample, the whole gather/mean/1x1-conv pipeline is reformulated as
  one data-dependent [256,256] @ [256,6400] matmul:
    rows   0..127 : G (+I)   -> x1[pos_idx] + x1          (tmp1 gather, x1-add folded)
    row    128    : negmask/128 (+e128)                   (mean of negatives + x1)
    rows 129..255 : Wg = W_pos @ G + outer(w_last, negmask/128)   (the 1x1 conv)
  followed by BN+LeakyReLU (one fused Lrelu ACT op with per-partition
  alpha/scale/bias; the mean row passes through via alpha=1) + x1-add.
  The 0/1 matrices are built on-device from the ECA channel scores via
  pairwise-comparison ranking (no sort, no data-dependent control flow).

  The big matmuls run in float32r (1 cycle/row on the PE at free-dim
  >=256, vs 4 cycles/row for plain fp32); the tiny ranking matmuls stay
  fp32 since reduced precision there could flip top-k membership of
  near-tied channels.

  build_nc(niter=N) wraps the whole per-iteration pipeline (x1 load ->
  ranking -> matmuls -> stores -> x0 passthrough) in a hardware For_i
  loop; test.py times niter=1 vs niter=N and reports the marginal
  per-iteration time, cancelling the ~87 ms axon-tunnel dispatch RTT
  that would otherwise swamp the ~0.2 ms kernel.
"""

import sys

sys.path.insert(0, "/opt/trn_rl_repo")

import numpy as np

import concourse.bass as bass
import concourse.tile as tile
from concourse import mybir
from concourse.bass_utils import run_bass_kernel_spmd

B, C, H, W = 16, 256, 80, 80
HALF = C // 2          # 128
NPIX = H * W           # 6400
NCORES = 8
SPC = B // NCORES      # 2 samples per core
NT = 512               # matmul n-tile (one PSUM bank of f32)
BN_EPS = 1e-5
F32 = mybir.dt.float32
F32R = mybir.dt.float32r


def build_nc(npix=NPIX, nsamp=SPC, niter=1):
    nc = bass.Bass("TRN2", target_bir_lowering=False, debug=False)

    def din(name, shape):
        return nc.dram_tensor(name, shape, F32, kind="ExternalInput").ap()

    x0 = din("x0", [nsamp, C, npix])
    x1 = din("x1", [nsamp, C, npix])
    wposT = din("wposT", [HALF, HALF - 1])      # W_pos^T
    wlastb = din("wlastb", [HALF, HALF - 1])    # w_last broadcast over partitions
    bnA = din("bnA", [HALF, 1])                 # [0]=1, [p]=gamma/sqrt(var+eps) for row p-1
    bnB = din("bnB", [HALF, 1])                 # [0]=0, [p]=beta-mean*bnA
    alphav = din("alphav", [HALF, 1])           # [0]=1 (identity), else 0.1 (lrelu slope)
    ecaw = din("ecaw", [1, 5])
    id128 = din("id128", [HALF, HALF])
    tri = din("tri", [HALF, HALF])              # tri[k,j] = 1 if k<j
    onesm = din("onesm", [HALF, HALF])
    iota = din("iota", [HALF, HALF])            # iota[j,p] = p
    ones1r = din("ones1r", [1, HALF])
    out = nc.dram_tensor("out", [nsamp, 2 * C, npix], F32, kind="ExternalOutput").ap()

    ntl = []
    o = 0
    while o < npix:
        ntl.append((o, min(NT, npix - o)))
        o += NT

    from contextlib import ExitStack
    with tile.TileContext(nc) as tc, ExitStack() as st:
        consts = st.enter_context(tc.tile_pool(name="consts", bufs=1))
        xin = st.enter_context(tc.tile_pool(name="xin", bufs=1))
        lhp = st.enter_context(tc.tile_pool(name="lhp", bufs=4))
        gtp = st.enter_context(tc.tile_pool(name="gtp", bufs=2))
        misc = st.enter_context(tc.tile_pool(name="misc", bufs=2))
        obp = st.enter_context(tc.tile_pool(name="obp", bufs=2))
        pbig = st.enter_context(tc.tile_pool(name="pbig", bufs=4, space="PSUM"))
        pmisc = st.enter_context(tc.tile_pool(name="pmisc", bufs=3, space="PSUM"))

        # ---- constants into SBUF (outside the timing loop) ----
        c_id = consts.tile([HALF, HALF], F32)
        c_tri = consts.tile([HALF, HALF], F32)
        c_ones = consts.tile([HALF, HALF], F32)
        c_iota = consts.tile([HALF, HALF], F32)
        c_1r = consts.tile([1, HALF], F32)
        c_wposT = consts.tile([HALF, HALF - 1], F32)
        c_wlastb = consts.tile([HALF, HALF - 1], F32)
        c_bnA = consts.tile([HALF, 1], F32)
        c_bnB = consts.tile([HALF, 1], F32)
        c_alpha = consts.tile([HALF, 1], F32)
        c_ecaw = consts.tile([1, 5], F32)
        for t, d in (
            (c_id, id128), (c_tri, tri), (c_ones, onesm), (c_iota, iota),
            (c_1r, ones1r), (c_wposT, wposT), (c_wlastb, wlastb),
            (c_bnA, bnA), (c_bnB, bnB), (c_alpha, alphav),
            (c_ecaw, ecaw),
        ):
            nc.gpsimd.dma_start(out=t, in_=d)

        def body():
            # ---- x1 into SBUF (stays resident for both passes) ----
            # All in-loop DMAs ride the two HWDGE rings: SWDGE (gpsimd)
            # DMAs inside a For_i loop emit InstIncSwdgeSem ISA ops that
            # this walrus cannot encode ("ISA wrong length").
            # Independent transfers (x1 loads, x0 passthrough) go on the
            # sync ring; the compute-dependent ob stores go on the scalar
            # ring so their semaphore waits can't head-of-line-block the
            # loads at the sequencer.
            X = [[None, None] for _ in range(nsamp)]
            for s in range(nsamp):
                for h in range(2):
                    t = xin.tile([HALF, npix], F32, tag=f"x_{s}_{h}")
                    nc.sync.dma_start(
                        out=t, in_=x1[s, h * HALF:(h + 1) * HALF, :])
                    X[s][h] = t

            # ---- phase A: channel sums -> row layout R [1, nsamp*(C+4)] ----
            R = misc.tile([1, nsamp * (C + 4)], F32, tag="R", bufs=1)
            nc.vector.memset(R, 0.0)
            for s in range(nsamp):
                for h in range(2):
                    sm = misc.tile([HALF, 1], F32, tag=f"sums_{s}_{h}", bufs=1)
                    nc.vector.reduce_sum(out=sm, in_=X[s][h],
                                         axis=mybir.AxisListType.X)
                    pr = pmisc.tile([1, HALF], F32, tag="mp")
                    nc.tensor.matmul(pr, sm, c_id, start=True, stop=True)
                    nc.scalar.copy(
                        out=R[0:1, s * (C + 4) + 2 + h * HALF:
                              s * (C + 4) + 2 + (h + 1) * HALF],
                        in_=pr)

            # ---- ECA conv k=5 along channels: Y [1, nsamp*C] ----
            Yr = misc.tile([1, nsamp, C], F32, tag="Y", bufs=1)
            Rr = R.rearrange("p (s c) -> p s c", s=nsamp)
            nc.vector.tensor_scalar(
                out=Yr, in0=Rr[:, :, 0:C], scalar1=c_ecaw[0:1, 0:1],
                scalar2=None, op0=mybir.AluOpType.mult)
            for k in range(1, 5):
                nc.vector.scalar_tensor_tensor(
                    out=Yr, in0=Rr[:, :, k:k + C], scalar=c_ecaw[0:1, k:k + 1],
                    in1=Yr, op0=mybir.AluOpType.mult, op1=mybir.AluOpType.add)

            # ---- per-sample: rank -> masks -> Lh ----
            LHS = [[None, None] for _ in range(nsamp)]
            for s in range(nsamp):
                pb = pmisc.tile([HALF, C], F32, tag="mp")
                nc.tensor.matmul(pb, c_1r, Yr[0:1, s, :], start=True, stop=True)
                m_sb, negdiv = [], []
                for h in range(2):
                    pyc = pmisc.tile([HALF, 1], F32, tag="mp")
                    nc.tensor.matmul(
                        pyc, Yr[0:1, s, h * HALF:(h + 1) * HALF],
                        c_1r[0:1, 0:1], start=True, stop=True)
                    yc = misc.tile([HALF, 1], F32, tag="yc")
                    nc.vector.tensor_copy(out=yc, in_=pyc)
                    cmp = misc.tile([HALF, C], F32, tag="cmp")
                    nc.vector.tensor_scalar(
                        out=cmp, in0=pb, scalar1=yc, scalar2=None,
                        op0=mybir.AluOpType.is_gt)
                    rankd = misc.tile([HALF, 1], F32, tag="rankd")
                    nc.vector.reduce_sum(out=rankd, in_=cmp,
                                         axis=mybir.AxisListType.X)
                    mh = misc.tile([HALF, 1], F32, tag="m")
                    nc.vector.tensor_scalar(
                        out=mh, in0=rankd, scalar1=float(HALF), scalar2=None,
                        op0=mybir.AluOpType.is_lt)
                    # nd = (1 - mh)/128  (mean-of-negatives divisor; also
                    # reused as the not-selected mask via nd*128)
                    nd = misc.tile([HALF, 1], F32, tag="nd")
                    nc.vector.tensor_scalar(
                        out=nd, in0=mh, scalar1=-1.0 / HALF,
                        scalar2=1.0 / HALF, op0=mybir.AluOpType.mult,
                        op1=mybir.AluOpType.add)
                    m_sb.append(mh); negdiv.append(nd)

                pr0 = pmisc.tile([HALF, 1], F32, tag="mp")
                nc.tensor.matmul(pr0, c_tri, m_sb[0], start=True, stop=True)
                pr1 = pmisc.tile([HALF, 1], F32, tag="mp")
                nc.tensor.matmul(pr1, c_ones, m_sb[0], start=True, stop=False)
                nc.tensor.matmul(pr1, c_tri, m_sb[1], start=False, stop=True)
                for h, prh in ((0, pr0), (1, pr1)):
                    rp = misc.tile([HALF, 1], F32, tag="rp")
                    nc.vector.scalar_tensor_tensor(
                        out=rp, in0=negdiv[h], scalar=256.0 * HALF, in1=prh,
                        op0=mybir.AluOpType.mult, op1=mybir.AluOpType.add)
                    gt_sb = gtp.tile([HALF, HALF], F32, tag="gt")
                    nc.vector.tensor_scalar(
                        out=gt_sb, in0=c_iota, scalar1=rp, scalar2=None,
                        op0=mybir.AluOpType.is_equal)
                    pgm = pmisc.tile([HALF, HALF], F32, tag="mp")
                    nc.tensor.matmul(pgm, gt_sb, c_id, start=True, stop=True)
                    gm_sb = gtp.tile([HALF, HALF], F32, tag="gm")
                    nc.vector.tensor_copy(out=gm_sb, in_=pgm)
                    pwg = pmisc.tile([HALF, HALF - 1], F32, tag="mp")
                    nc.tensor.matmul(pwg, gm_sb, c_wposT, start=True, stop=True)
                    lh = lhp.tile([HALF, C], F32, tag="lh")
                    if h == 0:
                        nc.vector.tensor_add(
                            out=lh[:, 0:HALF], in0=gt_sb, in1=c_id)
                    else:
                        nc.vector.tensor_copy(out=lh[:, 0:HALF], in_=gt_sb)
                    nc.vector.tensor_copy(
                        out=lh[:, HALF:HALF + 1], in_=negdiv[h])
                    nc.vector.scalar_tensor_tensor(
                        out=lh[:, HALF + 1:C], in0=c_wlastb, scalar=negdiv[h],
                        in1=pwg, op0=mybir.AluOpType.mult,
                        op1=mybir.AluOpType.add)
                    LHS[s][h] = lh
                # NOTE: no identity fold for inner row 128 — the mh1 epilogue
                # adds x1 for all 128 partitions (incl. the mean row at p=0).

            # f32r copies of the LHS matrices (fp32 matmul is 4 cycles/row
            # on the PE; f32r with free dim >=256 is 1 cycle/row). The DVE
            # copy is the sanctioned f32r-rounding producer.
            LHR = [[None, None] for _ in range(nsamp)]
            for s in range(nsamp):
                for h in range(2):
                    lhr = lhp.tile([HALF, C], F32R, tag="lhr")
                    nc.vector.tensor_copy(out=lhr, in_=LHS[s][h])
                    LHR[s][h] = lhr

            # ---- big matmuls + epilogue + stores ----
            # Loop tiles outermost so each x1 slice is rounded to f32r once
            # and feeds both output halves (mh=0 gather rows, mh=1 conv rows).
            SPL = 3072  # tile-aligned store split: write back first half early
            for s in range(nsamp):
                ob0 = obp.tile([HALF, npix], F32, tag="ob0", bufs=1)
                ob1 = obp.tile([HALF, npix], F32, tag="ob1", bufs=1)
                for (n0, nsz) in ntl:
                    xs = [None, None]
                    for h in range(2):
                        xs[h] = misc.tile([HALF, NT], F32R, tag=f"xs{h}",
                                          bufs=3, name=f"xs{h}")
                        nc.vector.tensor_copy(
                            out=xs[h][:, :nsz], in_=X[s][h][:, n0:n0 + nsz])
                    for mh in range(2):
                        ps = pbig.tile([HALF, NT], F32, tag="pb")
                        nc.tensor.matmul(
                            ps[:, :nsz],
                            LHR[s][0][:, mh * HALF:(mh + 1) * HALF],
                            xs[0][:, :nsz], start=True, stop=False)
                        nc.tensor.matmul(
                            ps[:, :nsz],
                            LHR[s][1][:, mh * HALF:(mh + 1) * HALF],
                            xs[1][:, :nsz], start=False, stop=True)
                        if mh == 0:
                            nc.vector.tensor_copy(
                                out=ob0[:, n0:n0 + nsz], in_=ps[:, :nsz])
                        else:
                            # BN + LeakyReLU(0.1) fused into one ACT op:
                            # prelu(ps*bnA + bnB; alpha) with alpha[0]=1 so the
                            # mean row (p=0) passes through unchanged. (Lrelu's
                            # alpha operand is ignored on this HW — fixed 0.01
                            # slope; Prelu honors the per-partition alpha AP.)
                            ta = misc.tile([HALF, NT], F32, tag="ta", bufs=3)
                            nc.scalar.activation(
                                out=ta[:, :nsz], in_=ps[:, :nsz],
                                func=mybir.ActivationFunctionType.Prelu,
                                bias=c_bnB, scale=c_bnA, alpha=c_alpha)
                            nc.vector.tensor_add(
                                out=ob1[:, n0:n0 + nsz], in0=ta[:, :nsz],
                                in1=X[s][1][:, n0:n0 + nsz])
                    if n0 + nsz == SPL:
                        nc.scalar.dma_start(
                            out=out[s, C:C + HALF, 0:SPL], in_=ob0[:, 0:SPL])
                        nc.scalar.dma_start(
                            out=out[s, C + HALF:2 * C, 0:SPL],
                            in_=ob1[:, 0:SPL])
                nc.scalar.dma_start(
                    out=out[s, C:C + HALF, SPL:npix], in_=ob0[:, SPL:npix])
                nc.scalar.dma_start(
                    out=out[s, C + HALF:2 * C, SPL:npix], in_=ob1[:, SPL:npix])
                # x0 passthrough DRAM->DRAM (sync HWDGE ring, no deps)
                nc.sync.dma_start(out=out[s, 0:C, :], in_=x0[s, :, :])

        if niter == 1:
            body()
        else:
            with tc.For_i(0, niter, 1):
                body()
    return nc


def _split_multiwait_drains(nc):
    """This container's walrus rejects >1 sync-wait on one instruction
    ("Too many sync wait commands" in setupSyncWait). Tile's kernel-tail
    Drain carries one wait per outstanding semaphore — split it into a
    chain of single-wait Drains."""
    for fn in nc.m.functions:
        for blk in fn.blocks:
            insts = list(blk.instructions)
            changed = False
            out = []
            for inst in insts:
                si = getattr(inst, "sync_info", None)
                waits = list(si.on_wait) if (si and si.on_wait) else []
                if len(waits) > 1:
                    for j, w in enumerate(waits[:-1]):
                        nd = mybir.InstEventSemaphore(
                            name=f"{inst.name}-sw{j}", ins=[], outs=[])
                        nd.engine = inst.engine
                        nd.sync_info = mybir.SyncInfo(
                            on_wait=[w], on_update=[])
                        out.append(nd)
                    si.on_wait = [waits[-1]]
                    changed = True
                out.append(inst)
            if changed:
                blk.instructions = out
    return nc


def host_consts(conv_w, bn_gamma, bn_beta, bn_mean, bn_var, eca_w):
    conv_w = np.asarray(conv_w, np.float32)
    a = (np.asarray(bn_gamma, np.float64)
         / np.sqrt(np.asarray(bn_var, np.float64) + BN_EPS))
    bnA = np.zeros((HALF, 1), np.float32)
    bnB = np.zeros((HALF, 1), np.float32)
    bnA[0, 0] = 1.0
    bnA[1:, 0] = a.astype(np.float32)
    bnB[1:, 0] = (np.asarray(bn_beta, np.float64)
                  - np.asarray(bn_mean, np.float64) * a).astype(np.float32)
    alphav = np.full((HALF, 1), 0.1, np.float32)
    alphav[0, 0] = 1.0
    return {
        "alphav": alphav,
        "wposT": np.ascontiguousarray(conv_w[:, :HALF].T),
        "wlastb": np.ascontiguousarray(
            np.tile(conv_w[:, HALF][None, :], (HALF, 1))),
        "bnA": bnA,
        "bnB": bnB,
        "ecaw": np.asarray(eca_w, np.float32).reshape(1, 5),
        "id128": np.eye(HALF, dtype=np.float32),
        "tri": np.triu(np.ones((HALF, HALF), np.float32), 1),
        "onesm": np.ones((HALF, HALF), np.float32),
        "iota": np.tile(np.arange(HALF, dtype=np.float32), (HALF, 1)),
        "ones1r": np.ones((1, HALF), np.float32),
    }


def kernel(x0, x1, eca_w, conv_w, bn_gamma, bn_beta, bn_mean, bn_var,
           _trace=False):
    x0 = np.asarray(x0, np.float32).reshape(B, C, NPIX)
    x1 = np.asarray(x1, np.float32).reshape(B, C, NPIX)
    cst = host_consts(conv_w, bn_gamma, bn_beta, bn_mean, bn_var, eca_w)
    nc = _split_multiwait_drains(build_nc())
    in_maps = []
    for c in range(NCORES):
        m = dict(cst)
        m["x0"] = np.ascontiguousarray(x0[c * SPC:(c + 1) * SPC])
        m["x1"] = np.ascontiguousarray(x1[c * SPC:(c + 1) * SPC])
        in_maps.append(m)
    res = run_bass_kernel_spmd(nc, in_maps, list(range(NCORES)), trace=False)
    out = np.concatenate([res.results[c]["out"] for c in range(NCORES)], axis=0)
    out = out.reshape(B, 2 * C, H, W)
    return out


def _make_jit_fn(nc):
    """jit(shard_map(bass_exec)) over 8 cores with donated zero output
    buffers, mirroring bass2jax.run_bass_via_pjrt but reusable."""
    import jax
    import jax.numpy as jnp
    from jax.sharding import Mesh, PartitionSpec
    from jax.experimental.shard_map import shard_map
    from concourse import bass2jax
    from concourse import mybir as _mb

    bass2jax.install_neuronx_cc_hook()
    pid_name = (nc.partition_id_tensor.name
                if nc.partition_id_tensor else None)
    in_names, out_names, out_avals, zero_shapes = [], [], [], []
    for alloc in nc.m.functions[0].allocations:
        if not isinstance(alloc, _mb.MemoryLocationSet):
            continue
        name = alloc.memorylocations[0].name
        if alloc.kind == "ExternalInput":
            if name != pid_name:
                in_names.append(name)
        elif alloc.kind == "ExternalOutput":
            out_names.append(name)
            shape = tuple(alloc.tensor_shape)
            dtype = _mb.dt.np(alloc.dtype)
            out_avals.append(jax.core.ShapedArray(shape, dtype))
            zero_shapes.append((shape, dtype))
    n_params = len(in_names)
    n_outs = len(out_names)
    all_names = list(in_names) + list(out_names) + (
        [pid_name] if pid_name else [])
    donate = tuple(range(n_params, n_params + n_outs))

    def _body(*args):
        operands = list(args)
        if pid_name:
            operands.append(bass2jax.partition_id_tensor())
        outs = bass2jax._bass_exec_p.bind(
            *operands, out_avals=tuple(out_avals), in_names=tuple(all_names),
            out_names=tuple(out_names), lowering_input_output_aliases=(),
            sim_require_finite=True, sim_require_nnan=True, nc=nc)
        return tuple(outs)

    devices = jax.devices()[:NCORES]
    mesh = Mesh(np.asarray(devices), ("core",))
    fn = jax.jit(shard_map(
        _body, mesh=mesh,
        in_specs=(PartitionSpec("core"),) * (n_params + n_outs),
        out_specs=(PartitionSpec("core"),) * n_outs,
        check_rep=False), donate_argnums=donate, keep_unused=True)
    sharding = jax.sharding.NamedSharding(mesh, PartitionSpec("core"))
    zfn = jax.jit(
        lambda: tuple(jnp.zeros((NCORES * sh[0], *sh[1:]), dt)
                      for (sh, dt) in zero_shapes),
        out_shardings=(sharding,) * n_outs)
    return fn, zfn, sharding, in_names, out_names


def _time_fn(fn, zfn, dev_in, iters=10, warmup=3):
    import time
    import jax
    times = []
    for i in range(warmup + iters):
        z = zfn()
        jax.block_until_ready(z)
        t0 = time.perf_counter()
        r = fn(*dev_in, *z)
        jax.block_until_ready(r)
        dt_s = time.perf_counter() - t0
        if i >= warmup:
            times.append(dt_s)
        del r
    times.sort()
    return times


NBIG = 513  # loop count of the timing NEFF


def bench(x0, x1, eca_w, conv_w, bn_gamma, bn_beta, bn_mean, bn_var,
          iters=14, warmup=3, nbig=NBIG):
    """Returns (output, per_iter_seconds, info). The per-call wall time
    through the axon tunnel is ~87 ms of dispatch RTT regardless of
    kernel content (a trivial copy kernel measures the same), so the
    kernel's HW execution time is measured differentially: a second
    NEFF runs the identical workload `nbig` times in a hardware For_i
    loop, and per-iteration time = (T(nbig) - T(1)) / (nbig - 1)."""
    import jax

    x0 = np.asarray(x0, np.float32).reshape(B, C, NPIX)
    x1 = np.asarray(x1, np.float32).reshape(B, C, NPIX)
    cst = host_consts(conv_w, bn_gamma, bn_beta, bn_mean, bn_var, eca_w)

    nc1 = _split_multiwait_drains(build_nc(niter=1))
    fn1, zfn1, sharding, in_names, out_names = _make_jit_fn(nc1)

    per_core = []
    for c in range(NCORES):
        m = dict(cst)
        m["x0"] = np.ascontiguousarray(x0[c * SPC:(c + 1) * SPC])
        m["x1"] = np.ascontiguousarray(x1[c * SPC:(c + 1) * SPC])
        per_core.append(m)
    concat_in = [np.concatenate([per_core[c][n] for c in range(NCORES)], axis=0)
                 for n in in_names]

    # correctness result from the niter=1 NEFF
    outs = fn1(*concat_in, *zfn1())
    jax.block_until_ready(outs)
    oidx = out_names.index("out")
    full = np.asarray(outs[oidx]).reshape(NCORES, SPC, 2 * C, NPIX)
    result = full.reshape(B, 2 * C, H, W)

    dev_in = [jax.device_put(a, sharding) for a in concat_in]

    ncN = _split_multiwait_drains(build_nc(niter=nbig))
    fnN, zfnN, _, _, _ = _make_jit_fn(ncN)
    outsN = fnN(*concat_in, *zfnN())
    jax.block_until_ready(outsN)
    fullN = np.asarray(outsN[oidx]).reshape(NCORES, SPC, 2 * C, NPIX)
    resultN = fullN.reshape(B, 2 * C, H, W)
    loop_dev = float(np.abs(resultN - result).max())

    # The tunnel RTT drifts by several ms on minute timescales, so the
    # 1-iter and nbig-iter dispatches are timed in interleaved pairs and
    # the marginal is taken per pair — drift cancels pairwise.
    import time as _time
    t1, tN, per_pair = [], [], []
    for i in range(warmup + iters):
        for fn_i, zfn_i, acc in ((fn1, zfn1, t1), (fnN, zfnN, tN)):
            z = zfn_i()
            jax.block_until_ready(z)
            t0 = _time.perf_counter()
            r = fn_i(*dev_in, *z)
            jax.block_until_ready(r)
            dt_s = _time.perf_counter() - t0
            if i >= warmup:
                acc.append(dt_s)
            del r
    per_pair = [(b - a) / (nbig - 1) for a, b in zip(t1, tN)]
    per_pair.sort()
    per_iter = per_pair[len(per_pair) // 2]
    t1s, tNs = sorted(t1), sorted(tN)
    info = {
        "t1": t1s, "tN": tNs, "med1": t1s[len(t1s) // 2],
        "medN": tNs[len(tNs) // 2], "per_pair": per_pair,
        "nbig": nbig, "loop_output_absdev": loop_dev,
    }
    return result, per_iter, info
